# revision 6
# baseline (speedup 1.0000x reference)
"""GAT (2-layer, DGL-style) Bass kernel v2 for 8 Trainium2 NeuronCores.

v2 design (vs baseline):
- Edge gathers via InstDMAGatherAnt (dma_gather): one instruction per
  (superblock, src-range) instead of one indirect DMA per 128 edges;
  descriptor generation drops ~10x on the Pool engine.
- Tables in bf16 rows sized to the 256B gather granularity:
  layer1 row (512B): [h bf16 0:128 | el bf16 128:136 | er bf16 136:144 | pad]
  layer2 row (256B): [h2 bf16 0:64 | ones-slot 64 | er2 bf16 65 | el2 bf16 66 | pad]
- Heads stored dim-major (col = d*HEADS + h) so the per-edge p broadcast is a
  packed-last-dim DVE op (2x/4x mode).
- Scatter via one-hot matmuls in bf16 (4x faster PE); one-hot built per-tile
  with tensor_scalar(iota, dstl-col, is_equal); layer2 folds p into the
  one-hot via the two-op tensor_scalar (is_equal then mult).
- er broadcast per edge via small PE matmuls with a transposed one-hot (OHT)
  built from a ones-matmul broadcast of dstlT + is_equal.
- Stage A (table build) replicated per core in bf16; h2 exchanged with one
  AllGather.

src indices are split into 4 ranges of 32768 (int16 gather index limit);
within each (window, range) the edge run is padded to a multiple of 128, with
the pad count shared across cores (SPMD single program).
"""

import numpy as np
import ml_dtypes
from contextlib import ExitStack
from dataclasses import dataclass

from concourse import bacc, bass, mybir, tile
from concourse.bass_utils import run_bass_kernel_spmd

f32 = mybir.dt.float32
bf16 = mybir.dt.bfloat16
i16 = mybir.dt.int16
i32 = mybir.dt.int32
ALU = mybir.AluOpType
ACTF = mybir.ActivationFunctionType
P = 128
RANGE = 32768
NBF = ml_dtypes.bfloat16


@dataclass
class Cfg:
    N: int = 100000
    E: int = 1600000
    IN: int = 256
    HID: int = 128
    HEADS: int = 8
    DH: int = 16
    NCLS: int = 64
    C: int = 8
    SLOPE: float = 0.2
    SBW: int = 2  # windows per superblock

    @property
    def S(self):
        return self.N // self.C

    @property
    def NW(self):
        return (self.S + P - 1) // P

    @property
    def NR(self):
        return (self.N + RANGE - 1) // RANGE

    @property
    def NSB(self):
        return (self.NW + self.SBW - 1) // self.SBW


class Plan:
    """Compile-time schedule, common to all cores."""

    def __init__(self, cfg: Cfg, kwr: np.ndarray):
        self.cfg = cfg
        self.kwr = kwr  # [NW, NR] tiles per (window, range)
        NW, NR, SBW = cfg.NW, cfg.NR, cfg.SBW
        self.sbs = []
        tile_pos = 0
        icol = 0
        for sb0 in range(0, NW, SBW):
            wins = list(range(sb0, min(sb0 + SBW, NW)))
            sb = {"wins": wins, "tile0": tile_pos, "ranges": [], "windows": {}}
            for w in wins:
                sb["windows"][w] = []  # list of (r, tile_start, ktiles)
            for r in range(NR):
                nt = int(sum(kwr[w, r] for w in wins))
                if nt == 0:
                    continue
                seg = {"r": r, "tile0": tile_pos, "ntiles": nt, "icol": icol}
                sb["ranges"].append(seg)
                t = tile_pos
                for w in wins:
                    k = int(kwr[w, r])
                    if k:
                        sb["windows"][w].append((r, t, k))
                        t += k
                tile_pos += nt
                icol += nt * 8  # nt*128/16
            sb["tiles"] = tile_pos - sb["tile0"]
            self.sbs.append(sb)
        self.TT = tile_pos
        self.ICOLS = icol


def host_prep(cfg: Cfg, src: np.ndarray, dst: np.ndarray):
    """Sort edges per core by (window, range); build shared plan + per-core
    idx/dstl blobs."""
    S, NW, NR, C = cfg.S, cfg.NW, cfg.NR, cfg.C
    src = np.asarray(src, np.int64)
    dst = np.asarray(dst, np.int64)
    shard = dst // S

    per_core = []
    cnts = np.zeros((C, NW, NR), np.int64)
    for c in range(C):
        m = shard == c
        s_c = src[m]
        dl = dst[m] - c * S
        w = dl >> 7
        r = s_c >> 15
        key = w * NR + r
        order = np.argsort(key, kind="stable")
        s_c, dl, key = s_c[order], dl[order], key[order]
        cnts[c] += np.bincount(key, minlength=NW * NR).reshape(NW, NR)
        per_core.append((s_c, dl, key))

    kwr = np.ceil(cnts.max(axis=0) / P).astype(np.int64)  # [NW, NR]
    plan = Plan(cfg, kwr)

    # per (w, r): global tile base
    tile_base = np.zeros((NW, NR), np.int64)
    icol_base = np.zeros((NW, NR), np.int64)
    for sb in plan.sbs:
        for seg in sb["ranges"]:
            t = seg["tile0"]
            ic = seg["icol"]
            for w in sb["wins"]:
                k = int(kwr[w, seg["r"]])
                if k:
                    tile_base[w, seg["r"]] = t
                    icol_base[w, seg["r"]] = ic
                    t += k
                    ic += k * 8

    TT, ICOLS = plan.TT, plan.ICOLS
    blobs = []
    for c in range(C):
        s_c, dl, key = per_core[c]
        dstl_blob = np.full((P, TT), -1.0, np.float32)
        dstlT_blob = np.full((TT * P,), -1.0, np.float32)
        idx_blob = np.zeros((P, ICOLS), np.int16)
        # bucket offsets within the sorted stream
        bucket_off = np.zeros(NW * NR + 1, np.int64)
        np.cumsum(cnts[c].reshape(-1), out=bucket_off[1:])
        for w in range(NW):
            for r in range(NR):
                n = int(cnts[c, w, r])
                if n == 0 and kwr[w, r] == 0:
                    continue
                o = bucket_off[w * NR + r]
                tb = tile_base[w, r]
                ib = icol_base[w, r]
                k = int(kwr[w, r])
                npad = k * P
                idx_rel = np.zeros(npad, np.int16)
                dl_pad = np.full(npad, -1.0, np.float32)
                if n:
                    idx_rel[:n] = (s_c[o:o + n] - r * RANGE).astype(np.int16)
                    dl_pad[:n] = (dl[o:o + n] - w * P).astype(np.float32)
                # dstl: position q -> tile tb + q//128, partition q%128
                dstl_blob[:, tb:tb + k] = dl_pad.reshape(k, P).T
                dstlT_blob[tb * P:(tb + k) * P] = dl_pad
                # idx: gather-local position q2 (same q since runs are packed
                # consecutively within the (sb, r) block): wrapped 16
                # local block col = ib + q//16, partition q%16 (replicated x8)
                cols = ib + np.arange(npad) // 16
                parts = np.arange(npad) % 16
                for g in range(8):
                    idx_blob[16 * g + parts, cols] = idx_rel
        blobs.append({
            "idx_blob": idx_blob,
            "dstl_blob": dstl_blob,
            "dstlT_blob": dstlT_blob.astype(NBF).reshape(1, TT * P),
        })
    return plan, blobs


def build_program(cfg: Cfg, plan: Plan, repeat: int = 1, debug_stage: int = 0):
    nc = bacc.Bacc("TRN2", target_bir_lowering=False, debug=False,
                   enable_asserts=False, num_devices=cfg.C)
    S, NW, NR, SBW = cfg.S, cfg.NW, cfg.NR, cfg.SBW
    IN, HID, HEADS, DH, NCLS = cfg.IN, cfg.HID, cfg.HEADS, cfg.DH, cfg.NCLS
    TT, ICOLS = plan.TT, plan.ICOLS
    ROW1 = 256   # bf16 cols per table1 row (512B)
    ROW2 = 128   # bf16 cols per table2 row (256B)
    F1 = HID + 2 * HEADS  # stage-A matmul width: h | el | er

    # ---- I/O ----
    xT_d = nc.dram_tensor("xT", [IN, cfg.N], bf16, kind="ExternalInput").ap()
    W1e_d = nc.dram_tensor("W1ext", [IN, F1], bf16, kind="ExternalInput").ap()
    W2e_d = nc.dram_tensor("W2ext", [HID, NCLS + 2], bf16, kind="ExternalInput").ap()
    idx_d = nc.dram_tensor("idx_blob", [P, ICOLS], i16, kind="ExternalInput").ap()
    dstl_d = nc.dram_tensor("dstl_blob", [P, TT], f32, kind="ExternalInput").ap()
    dstlT_d = nc.dram_tensor("dstlT_blob", [1, TT * P], bf16, kind="ExternalInput").ap()
    out_d = nc.dram_tensor("out", [S, NCLS], f32, kind="ExternalOutput").ap()

    table1_d = nc.dram_tensor("table1", [cfg.N, ROW1], bf16).ap()
    h2sh_d = nc.dram_tensor("h2sh", [S, ROW2], bf16).ap()
    h2full_d = nc.dram_tensor("h2full", [cfg.N, ROW2], bf16, addr_space="Shared").ap()

    ranges = []
    for r in range(cfg.NR):
        r0 = r * RANGE
        r1 = min(cfg.N, (r + 1) * RANGE)
        ranges.append((r0, r1))

    with tile.TileContext(nc) as tc, ExitStack() as octx:
        const = octx.enter_context(tc.tile_pool(name="const", bufs=1))

        iota_i = const.tile([P, P], i32)
        nc.gpsimd.iota(iota_i[:], pattern=[[1, P]], base=0, channel_multiplier=0)
        iota_r = const.tile([P, P], bf16)
        nc.vector.tensor_copy(iota_r[:], iota_i[:])
        iota_ci = const.tile([P, 1], i32)
        nc.gpsimd.iota(iota_ci[:], pattern=[[0, 1]], base=0, channel_multiplier=1)
        iota_c = const.tile([P, 1], f32)
        nc.vector.tensor_copy(iota_c[:], iota_ci[:])
        ones_row = const.tile([1, P], bf16)
        nc.vector.memset(ones_row[:], 1.0)
        negone = const.tile([P, 1], f32)
        nc.vector.memset(negone[:], -1.0)
        from concourse.masks import make_identity
        ident = const.tile([P, P], bf16)
        make_identity(nc, ident[:])
        W2e = const.tile([P, NCLS + 2], bf16)
        nc.sync.dma_start(out=W2e[:HID, :], in_=W2e_d[:, :])

        sbase = nc.partition_id() * S

        # ================= stage A: table1 (replicated) =================
        def stage_a(actx: ExitStack):
            wp = actx.enter_context(tc.tile_pool(name="a_w", bufs=1))
            KC = IN // P
            w1e = []
            for kc in range(KC):
                t = wp.tile([P, F1], bf16, tag=f"w1e{kc}")
                nc.sync.dma_start(out=t[:], in_=W1e_d[kc * P:(kc + 1) * P, :])
                w1e.append(t)
            xp = actx.enter_context(tc.tile_pool(name="a_x", bufs=3))
            stp = actx.enter_context(tc.tile_pool(name="a_st", bufs=3))
            psp = actx.enter_context(tc.tile_pool(name="a_ps", bufs=2, space="PSUM"))
            GA = 512
            for g0 in range(0, cfg.N, GA):
                gw = min(GA, cfg.N - g0)
                gt = (gw + P - 1) // P
                xa = xp.tile([P, KC * GA], bf16, tag="xa")
                for kc in range(KC):
                    nc.sync.dma_start(
                        out=xa[:, kc * GA:kc * GA + gw],
                        in_=xT_d[kc * P:(kc + 1) * P, g0:g0 + gw])
                st = stp.tile([P, GA // P * ROW1], bf16, tag="ast")
                ps = psp.tile([P, gt * 512], f32, tag="aps")
                for t in range(gt):
                    tw = min(P, gw - t * P)
                    for kc in range(KC):
                        nc.tensor.matmul(
                            ps[:tw, t * 512:t * 512 + F1],
                            lhsT=xa[:, kc * GA + t * P:kc * GA + t * P + tw],
                            rhs=w1e[kc][:],
                            start=(kc == 0), stop=(kc == KC - 1))
                # h (dim-major) -> bf16 cols [t*256, +128)
                nc.scalar.activation(
                    st[:, 0:gt * ROW1].rearrange("p (t c) -> p t c", c=ROW1)[:, :, 0:HID],
                    ps[:, 0:gt * 512].rearrange("p (t c) -> p t c", c=512)[:, :, 0:HID],
                    ACTF.Copy)
                # el|er -> bf16 cols [t*256+128, +16)
                nc.vector.tensor_copy(
                    st[:, 0:gt * ROW1].rearrange("p (t c) -> p t c", c=ROW1)[:, :, HID:HID + 16],
                    ps[:, 0:gt * 512].rearrange("p (t c) -> p t c", c=512)[:, :, HID:HID + 16])
                if gw == GA:
                    nc.sync.dma_start(
                        out=table1_d[g0:g0 + gw, :].rearrange(
                            "(t p) c -> p t c", p=P),
                        in_=st[:, 0:gt * ROW1].rearrange("p (t c) -> p t c", c=ROW1))
                else:
                    for t in range(gt):
                        tw = min(P, gw - t * P)
                        nc.sync.dma_start(
                            out=table1_d[g0 + t * P:g0 + t * P + tw, :],
                            in_=st[:tw, t * ROW1:(t + 1) * ROW1])

        # ================= edge phase =================
        def edge_phase(layer: int, ectx: ExitStack):
            if layer == 1:
                TBL, ROW, NH, MW = table1_d, ROW1, HEADS, HID + HEADS
                ERCOL = 136
            else:
                TBL, ROW, NH, MW = h2full_d, ROW2, 1, NCLS + 1
                ERCOL = 65
            ip = ectx.enter_context(tc.tile_pool(name=f"e{layer}_i", bufs=2))
            gp = ectx.enter_context(tc.tile_pool(name=f"e{layer}_g", bufs=2))
            ohtp = ectx.enter_context(tc.tile_pool(name=f"e{layer}_oht", bufs=2))
            ohp = ectx.enter_context(tc.tile_pool(name=f"e{layer}_oh", bufs=4))
            ohap = ectx.enter_context(tc.tile_pool(name=f"e{layer}_oha", bufs=2))
            sp = ectx.enter_context(tc.tile_pool(name=f"e{layer}_s", bufs=3))
            wp = ectx.enter_context(tc.tile_pool(name=f"e{layer}_w", bufs=3))
            stp = ectx.enter_context(tc.tile_pool(name=f"e{layer}_st", bufs=2))
            accp = ectx.enter_context(tc.tile_pool(name=f"e{layer}_acc", bufs=2, space="PSUM"))
            psep = ectx.enter_context(tc.tile_pool(name=f"e{layer}_pse", bufs=2, space="PSUM"))
            dxp = ectx.enter_context(tc.tile_pool(name=f"e{layer}_dx", bufs=2, space="PSUM"))
            if layer == 1:
                ptp = ectx.enter_context(tc.tile_pool(name="e1_pt", bufs=1, space="PSUM"))

            for sbi, sb in enumerate(plan.sbs):
                T_sb = sb["tiles"]
                tile0 = sb["tile0"]
                nwsb = len(sb["wins"])
                if T_sb == 0:
                    continue

                idx_t = ip.tile([P, T_sb * 8], i16, tag="idx")
                nc.sync.dma_start(
                    out=idx_t[:],
                    in_=idx_d[:, sb["ranges"][0]["icol"]:sb["ranges"][0]["icol"] + T_sb * 8])
                dstl_t = ip.tile([P, T_sb], f32, tag="dstl")
                nc.sync.dma_start(out=dstl_t[:], in_=dstl_d[:, tile0:tile0 + T_sb])
                dstlT_t = ip.tile([1, T_sb * P], bf16, tag="dstlT")
                nc.sync.dma_start(
                    out=dstlT_t[:],
                    in_=bass.AP(tensor=dstlT_d.tensor, offset=tile0 * P,
                                ap=[[1, 1], [1, T_sb * P]]))

                # erwin per window (own-shard rows of TBL/h2sh)
                erw = sp.tile([P, nwsb * NH], bf16, tag="erw")
                for wi, w in enumerate(sb["wins"]):
                    base = w * P
                    ns = min(P, S - base)
                    if ns <= 0:
                        continue
                    if layer == 1:
                        src_ap = TBL[bass.ds(sbase + base, ns), ERCOL:ERCOL + NH]
                    else:
                        src_ap = h2sh_d[base:base + ns, ERCOL:ERCOL + NH]
                    if ns < P:
                        nc.vector.memset(erw[:, wi * NH:(wi + 1) * NH], 0.0)
                    nc.sync.dma_start(out=erw[:ns, wi * NH:(wi + 1) * NH], in_=src_ap)

                # gathers (one per range, each into its own tile to keep
                # per-partition extents under the AP partition-step limit)
                Gt = {}  # r -> (tile, range-local tile0)
                for seg in sb["ranges"]:
                    r0, r1 = ranges[seg["r"]]
                    nt = seg["ntiles"]
                    nidx = nt * P
                    Gr = gp.tile([P, nt * ROW], bf16, tag=f"G{seg['r']}")
                    ic = seg["icol"] - sb["ranges"][0]["icol"]
                    nc.gpsimd.dma_gather(
                        out_ap=bass.AP(
                            tensor=Gr[:].tensor,
                            offset=Gr[:].offset,
                            ap=[list(Gr[:].ap[0]), [ROW, nt], [1, ROW]]),
                        in_ap=TBL[r0:r1, :],
                        idxs_ap=idx_t[:, ic:ic + nt * 8],
                        num_idxs=nidx, num_idxs_reg=nidx, elem_size=ROW,
                        single_packet=False)
                    Gt[seg["r"]] = (Gr, seg["tile0"] - tile0)
                    if layer == 2:
                        nc.vector.memset(
                            bass.AP(tensor=Gr[:].tensor, offset=Gr[:].offset + 64,
                                    ap=[list(Gr[:].ap[0]), [ROW, nt], [1, 1]]), 1.0)

                # OHT: dx broadcast + is_equal (alternate DVE-direct / ACT+DVE)
                OHT = ohtp.tile([P, T_sb * P], bf16, tag="OHT")
                CH = 512
                for ci, c0 in enumerate(range(0, T_sb * P, CH)):
                    cw = min(CH, T_sb * P - c0)
                    dx = dxp.tile([P, CH], f32, tag="dx")
                    nc.tensor.matmul(dx[:, :cw], lhsT=ones_row[:],
                                     rhs=dstlT_t[:, c0:c0 + cw], start=True, stop=True)
                    if ci % 2 == 0:
                        nc.vector.tensor_scalar(
                            out=OHT[:, c0:c0 + cw], in0=dx[:, :cw],
                            scalar1=iota_c[:, 0:1], scalar2=None, op0=ALU.is_equal)
                    else:
                        dxb = sp.tile([P, CH], bf16, tag="dxb")
                        nc.scalar.activation(dxb[:, :cw], dx[:, :cw], ACTF.Copy)
                        nc.vector.tensor_scalar(
                            out=OHT[:, c0:c0 + cw], in0=dxb[:, :cw],
                            scalar1=iota_c[:, 0:1], scalar2=None, op0=ALU.is_equal)

                # one-hot for every tile of the sb in one op:
                # OH_all[p, (t, s)] = (dstl[p, t] == s)
                OH_all = ohap.tile([P, T_sb * P], bf16, tag="OHall")
                nc.vector.tensor_tensor(
                    out=OH_all[:],
                    in0=bass.AP(tensor=iota_r[:].tensor, offset=iota_r[:].offset,
                                ap=[list(iota_r[:].ap[0]), [0, T_sb], [1, P]]),
                    in1=bass.AP(tensor=dstl_t[:].tensor, offset=dstl_t[:].offset,
                                ap=[list(dstl_t[:].ap[0]), [1, T_sb], [0, P]]),
                    op=ALU.is_equal)

                # stage tiles for this sb
                if layer == 1:
                    h2st = stp.tile([P, nwsb * ROW2], bf16, tag="h2st")
                else:
                    ost = stp.tile([P, nwsb * NCLS], f32, tag="ost")

                for wi, w in enumerate(sb["wins"]):
                    segs = sb["windows"][w]
                    k_w = sum(k for (_, _, k) in segs)
                    base = w * P
                    ns = min(P, S - base)
                    if k_w == 0:
                        if layer == 1:
                            nc.vector.memset(h2st[:, wi * ROW2:(wi + 1) * ROW2], 0.0)
                        else:
                            nc.vector.memset(ost[:, wi * NCLS:(wi + 1) * NCLS], 0.0)
                        continue
                    erw_w = erw[:, wi * NH:(wi + 1) * NH]

                    # pse: er per edge  [128, k_w*NH]
                    pse = psep.tile([P, k_w * NH], f32, tag="pse")
                    j = 0
                    for (r, t0, k) in segs:
                        for t in range(t0, t0 + k):
                            tl = t - tile0
                            nc.tensor.matmul(
                                pse[:, j * NH:(j + 1) * NH],
                                lhsT=OHT[:, tl * P:(tl + 1) * P],
                                rhs=erw_w, start=True, stop=True)
                            j += 1

                    # et = el + pse ; p = max(exp(et), exp(slope*et)) -> G
                    et = wp.tile([P, k_w * NH], f32, tag="et")
                    j = 0
                    for (r, t0, k) in segs:
                        Gr, rt0 = Gt[r]
                        tl = t0 - tile0 - rt0
                        if layer == 1:
                            el_view = bass.AP(
                                tensor=Gr[:].tensor, offset=Gr[:].offset + tl * ROW + 128,
                                ap=[list(Gr[:].ap[0]), [ROW, k], [1, 8]])
                        else:
                            el_view = bass.AP(
                                tensor=Gr[:].tensor, offset=Gr[:].offset + tl * ROW + 66,
                                ap=[list(Gr[:].ap[0]), [ROW, k], [1, 1]])
                        nc.vector.tensor_tensor(
                            out=et[:, j * NH:(j + k) * NH], in0=el_view,
                            in1=pse[:, j * NH:(j + k) * NH], op=ALU.add)
                        j += k
                    ea = wp.tile([P, k_w * NH], f32, tag="ea")
                    nc.scalar.activation(ea[:], et[:], ACTF.Exp)
                    eb = wp.tile([P, k_w * NH], f32, tag="eb")
                    nc.scalar.activation(eb[:], et[:], ACTF.Exp, scale=cfg.SLOPE)
                    if layer == 1:
                        j = 0
                        for (r, t0, k) in segs:
                            Gr, rt0 = Gt[r]
                            tl = t0 - tile0 - rt0
                            p_view = bass.AP(
                                tensor=Gr[:].tensor, offset=Gr[:].offset + tl * ROW + 128,
                                ap=[list(Gr[:].ap[0]), [ROW, k], [1, 8]])
                            nc.vector.tensor_tensor(
                                out=p_view, in0=ea[:, j * 8:(j + k) * 8],
                                in1=eb[:, j * 8:(j + k) * 8], op=ALU.max)
                            j += k
                        # msg = h * p (dim-major head broadcast)
                        j = 0
                        for (r, t0, k) in segs:
                            Gr, rt0 = Gt[r]
                            tl = t0 - tile0 - rt0
                            h_view = bass.AP(
                                tensor=Gr[:].tensor, offset=Gr[:].offset + tl * ROW,
                                ap=[list(Gr[:].ap[0]), [ROW, k], [8, 16], [1, 8]])
                            pb_view = bass.AP(
                                tensor=Gr[:].tensor, offset=Gr[:].offset + tl * ROW + 128,
                                ap=[list(Gr[:].ap[0]), [ROW, k], [0, 16], [1, 8]])
                            nc.vector.tensor_tensor(
                                out=h_view, in0=h_view, in1=pb_view, op=ALU.mult)
                            j += k
                    else:
                        p2 = wp.tile([P, k_w], f32, tag="p2")
                        nc.vector.tensor_tensor(out=p2[:], in0=ea[:], in1=eb[:],
                                                op=ALU.max)

                    # scatter
                    acc = accp.tile([P, MW], f32, tag="acc")
                    j = 0
                    first = True
                    for (r, t0, k) in segs:
                        Gr, rt0 = Gt[r]
                        for t in range(t0, t0 + k):
                            tl = t - tile0
                            tlr = t - tile0 - rt0
                            if layer == 1:
                                lhs = OH_all[:, tl * P:(tl + 1) * P]
                            else:
                                OH = ohp.tile([P, P], bf16, tag="OH")
                                nc.vector.tensor_scalar(
                                    out=OH[:], in0=OH_all[:, tl * P:(tl + 1) * P],
                                    scalar1=p2[:, j:j + 1], scalar2=None,
                                    op0=ALU.mult)
                                lhs = OH[:]
                            nc.tensor.matmul(
                                acc[:, :],
                                lhsT=lhs,
                                rhs=Gr[:, tlr * ROW:tlr * ROW + MW],
                                start=first, stop=(j == k_w - 1))
                            first = False
                            j += 1

                    # normalize + nonlinearity + output row build
                    scl = wp.tile([P, NH], f32, tag="scl")
                    nc.vector.tensor_scalar(out=scl[:], in0=acc[:, MW - NH:MW],
                                            scalar1=1e-30, scalar2=None, op0=ALU.max)
                    rs = wp.tile([P, NH], f32, tag="rs")
                    nc.vector.reciprocal(rs[:], scl[:])

                    if layer == 1:
                        h1 = wp.tile([P, HID], f32, tag="h1")
                        nc.vector.tensor_tensor(
                            out=h1[:].rearrange("p (d h) -> p d h", h=8),
                            in0=acc[:, 0:HID].rearrange("p (d h) -> p d h", h=8),
                            in1=bass.AP(tensor=rs[:].tensor, offset=rs[:].offset,
                                        ap=[list(rs[:].ap[0]), [0, 16], [1, 8]]),
                            op=ALU.mult)
                        # fused elu(elu(x)) = max(x, exp(exp(min(x,0)) - 1) - 1)
                        tmin = wp.tile([P, HID], f32, tag="tmin")
                        nc.vector.tensor_scalar(out=tmin[:], in0=h1[:],
                                                scalar1=0.0, scalar2=None, op0=ALU.min)
                        e1 = wp.tile([P, HID], f32, tag="e1")
                        nc.scalar.activation(e1[:], tmin[:], ACTF.Exp)
                        e2 = wp.tile([P, HID], f32, tag="e2")
                        nc.scalar.activation(e2[:], e1[:], ACTF.Exp, bias=negone[:, 0:1])
                        em1 = wp.tile([P, HID], f32, tag="em1")
                        nc.vector.tensor_scalar(out=em1[:], in0=e2[:],
                                                scalar1=-1.0, scalar2=None, op0=ALU.add)
                        z = wp.tile([P, HID], f32, tag="z")
                        nc.vector.tensor_tensor(out=z[:], in0=h1[:], in1=em1[:],
                                                op=ALU.max)
                        zb = wp.tile([P, HID], bf16, tag="zb")
                        nc.scalar.activation(zb[:], z[:], ACTF.Copy)
                        pt = ptp.tile([P, P], bf16, tag="pt")
                        nc.tensor.transpose(pt[:], zb[:], ident[:])
                        zT = wp.tile([P, P], bf16, tag="zT")
                        nc.scalar.activation(zT[:], pt[:], ACTF.Copy)
                        h2ps = ptp.tile([P, NCLS + 2], f32, tag="h2ps")
                        nc.tensor.matmul(h2ps[:], lhsT=zT[:HID, :], rhs=W2e[:HID, :],
                                         start=True, stop=True)
                        # row: [h2 bf16 0:64 | slot64=0 | er2 bf16 65 | el2 66]
                        c0 = wi * ROW2
                        nc.scalar.activation(
                            h2st[:, c0:c0 + NCLS], h2ps[:, 0:NCLS], ACTF.Copy)
                        nc.vector.memset(h2st[:, c0 + 64:c0 + 65], 0.0)
                        # [el2 | er2] -> cols 66, 65: copy er2 then el2
                        nc.vector.tensor_copy(
                            h2st[:, c0 + 65:c0 + 66], h2ps[:, NCLS + 1:NCLS + 2])
                        nc.vector.tensor_copy(
                            h2st[:, c0 + 66:c0 + 67], h2ps[:, NCLS:NCLS + 1])
                    else:
                        nc.vector.tensor_scalar(
                            out=ost[:, wi * NCLS:(wi + 1) * NCLS],
                            in0=acc[:, 0:NCLS], scalar1=rs[:, 0:1], scalar2=None,
                            op0=ALU.mult)

                # write stage per sb
                w0 = sb["wins"][0]
                rows0 = w0 * P
                rows1 = min(S, (sb["wins"][-1] + 1) * P)
                nfull = (rows1 - rows0) // P
                rem = (rows1 - rows0) % P
                if layer == 1:
                    if nfull:
                        nc.sync.dma_start(
                            out=h2sh_d[rows0:rows0 + nfull * P, :].rearrange(
                                "(t p) c -> p t c", p=P),
                            in_=h2st[:, 0:nfull * ROW2].rearrange(
                                "p (t c) -> p t c", c=ROW2))
                    if rem:
                        nc.sync.dma_start(
                            out=h2sh_d[rows0 + nfull * P:rows1, :],
                            in_=h2st[:rem, nfull * ROW2:(nfull + 1) * ROW2])
                else:
                    if nfull:
                        nc.sync.dma_start(
                            out=out_d[rows0:rows0 + nfull * P, :].rearrange(
                                "(t p) c -> p t c", p=P),
                            in_=ost[:, 0:nfull * NCLS].rearrange(
                                "p (t c) -> p t c", c=NCLS))
                    if rem:
                        nc.sync.dma_start(
                            out=out_d[rows0 + nfull * P:rows1, :],
                            in_=ost[:rem, nfull * NCLS:(nfull + 1) * NCLS])

        for _rep in range(repeat):
            with nc.named_scope("stageA"), ExitStack() as actx:
                stage_a(actx)
            if debug_stage >= 2 or debug_stage == 0:
                with nc.named_scope("edge1"), ExitStack() as e1ctx:
                    edge_phase(1, e1ctx)
            if debug_stage >= 3 or debug_stage == 0:
                with nc.named_scope("allgather"):
                    nc.gpsimd.collective_compute(
                        "AllGather", ALU.bypass, replica_groups=[list(range(cfg.C))],
                        ins=[h2sh_d[:, :]], outs=[h2full_d[:, :]])
            if debug_stage == 0:
                with nc.named_scope("edge2"), ExitStack() as e2ctx:
                    edge_phase(2, e2ctx)
        if debug_stage != 0:
            with ExitStack() as dctx:
                dp = dctx.enter_context(tc.tile_pool(name="dbg", bufs=1))
                for w0 in range(0, S, P):
                    ns = min(P, S - w0)
                    dt_ = dp.tile([P, NCLS], f32, tag="d")
                    nc.vector.memset(dt_[:], 0.0)
                    nc.sync.dma_start(out=out_d[w0:w0 + ns, :], in_=dt_[:ns, :])

    nc.compile()
    return nc


def make_inmaps(cfg: Cfg, inputs: dict, blobs):
    x = np.asarray(inputs["x"], np.float32)
    W1 = np.asarray(inputs["W1"], np.float32)
    al1 = np.asarray(inputs["al1"], np.float32)
    ar1 = np.asarray(inputs["ar1"], np.float32)
    W2 = np.asarray(inputs["W2"], np.float32)
    al2 = np.asarray(inputs["al2"], np.float32)
    ar2 = np.asarray(inputs["ar2"], np.float32)
    H, D = cfg.HEADS, cfg.DH

    xT = np.ascontiguousarray(x.T).astype(NBF)
    # dim-major: new col (d*H + h) = old col (h*D + d)
    newcols = np.zeros(cfg.HID, np.int64)
    for d in range(D):
        for h in range(H):
            newcols[d * H + h] = h * D + d
    W1p = W1[:, newcols]
    # el_h = sum_d h_dm[d*H+h] * al1[h, d]
    almat = np.zeros((cfg.HID, H), np.float32)
    armat = np.zeros((cfg.HID, H), np.float32)
    for h in range(H):
        for d in range(D):
            almat[d * H + h, h] = al1[h, d]
            armat[d * H + h, h] = ar1[h, d]
    W1el = W1p @ almat
    W1er = W1p @ armat
    W1ext = np.concatenate([W1p, W1el, W1er], axis=1).astype(NBF)
    # W2 rows must be dim-major-permuted to match z layout
    W2p = W2[newcols, :]
    W2ext = np.concatenate([W2p, W2p @ al2[0][:, None], W2p @ ar2[0][:, None]],
                           axis=1).astype(NBF)

    in_maps = []
    for c in range(cfg.C):
        b = blobs[c]
        in_maps.append({
            "xT": xT, "W1ext": W1ext, "W2ext": W2ext,
            "idx_blob": b["idx_blob"], "dstl_blob": b["dstl_blob"],
            "dstlT_blob": b["dstlT_blob"],
        })
    return in_maps


def run(cfg: Cfg, inputs: dict, trace: bool = False):
    plan, blobs = host_prep(cfg, inputs["src"], inputs["dst"])
    nc = build_program(cfg, plan)
    in_maps = make_inmaps(cfg, inputs, blobs)
    res = run_bass_kernel_spmd(nc, in_maps, core_ids=list(range(cfg.C)), trace=trace)
    out = np.concatenate([res.results[c]["out"] for c in range(cfg.C)], axis=0)
    return out, res


def kernel(**inputs) -> np.ndarray:
    cfg = Cfg()
    out, _ = run(cfg, inputs)
    return out.astype(np.float32)
